# revision 1
# baseline (speedup 1.0000x reference)
"""GateRow kernel for Trainium2 (8 NeuronCores, SPMD data-parallel over batch).

Problem: out[b, g] = gates[g, 2*x[b, c0[g]] + x[b, c1[g]]]
  x: [16384, 8192] bool, gates: [8192, 4] bool, choices: [8192, 2] int32.

Strategy (per core, batch-sharded BS=2048):
  host:  build a doubled lookup table TAB = [x^T ; ~x^T ; ones ; zeros]
         (uint8, one row per input wire, BS bytes per row).  Classify each
         gate's 4-entry truth table into  out = (s>=t1) ^ (s>=t2)  with
         s = va + vb, where va/vb are the (possibly inverted / constant)
         gathered operand rows.  This covers all 16 boolean functions.
  device:
    1. dma_gather rows of TAB -> operand tiles [128 gates, BS] uint8
    2. one fused custom-DVE pass: l = (a+b >= t1) ^ (a+b >= t2) -> bf16
    3. PE transpose (identity matmul) [128,128] tiles -> PSUM f32
    4. ACT copies PSUM -> SBUF uint8 (cast)
    5. DMA out rows [b, g] (contiguous per batch row)
"""

import sys

for _p in ("/opt/trn_rl_repo", "/opt/pypackages"):
    if _p not in sys.path:
        sys.path.append(_p)

from contextlib import ExitStack

import numpy as np
import ml_dtypes

import concourse.bass as bass
import concourse.bacc as bacc
import concourse.tile as tile
import concourse.mybir as mybir
from concourse.bass_utils import run_bass_kernel_spmd

B, N, G, NCORES = 16384, 8192, 8192, 8
BS = B // NCORES  # 2048 batch rows per core

# ---------------------------------------------------------------------------
# Gate classification: truth table (4 bits, bit (2a+b)) ->
#   (fa, fb, t1, t2) with fa/fb in {0: v, 1: ~v, 2: one, 3: zero}
#   such that f(a,b) == ((va+vb) >= t1) ^ ((va+vb) >= t2)
# ---------------------------------------------------------------------------


def _classify_gates():
    forms = np.zeros((16, 4), dtype=np.int64)
    for tt in range(16):
        found = False
        for fa in range(4):
            for fb in range(4):
                for t1 in range(4):
                    for t2 in range(4):
                        ok = True
                        for a in (0, 1):
                            for b in (0, 1):
                                va = (a, 1 - a, 1, 0)[fa]
                                vb = (b, 1 - b, 1, 0)[fb]
                                s = va + vb
                                v = int(s >= t1) ^ int(s >= t2)
                                if v != ((tt >> (2 * a + b)) & 1):
                                    ok = False
                        if ok and not found:
                            forms[tt] = (fa, fb, t1, t2)
                            found = True
        assert found, f"truth table {tt} not representable"
    return forms


_FORMS = _classify_gates()

# ---------------------------------------------------------------------------
# Custom DVE op:  out = ((in0+in1) >= s0) ^ ((in0+in1) >= s1)
# ---------------------------------------------------------------------------

_GATE_LUT_OP = None


def _register_gate_lut():
    global _GATE_LUT_OP
    if _GATE_LUT_OP is not None:
        return _GATE_LUT_OP
    import concourse.dve_ops as dve_ops_mod
    from concourse.dve_ops import DveOp
    from concourse.dve_spec import Spec, Src0, Src1, C0, C1, lower, _has_src1
    from concourse.dve_uop import DveOpSpec

    name = "GATE_LUT_ANT"
    if any(op.name == name for op in dve_ops_mod.OPS):
        _GATE_LUT_OP = next(op for op in dve_ops_mod.OPS if op.name == name)
        return _GATE_LUT_OP

    s = Src0 + Src1
    spec = Spec(
        body=(s >= C0) ^ (s >= C1),
        reference=lambda in0, in1, s0, s1, imm2: (
            ((in0 + in1) >= s0) != ((in0 + in1) >= s1)
        ).astype(np.float32),
    )
    row = dve_ops_mod._CUSTOM_DVE_ROW_BASE + len(dve_ops_mod.OPS)
    dve_ops_mod._SUB_OPCODE_FOR_NAME[name] = row
    shas = {}
    for ver in ("v3", "v4"):
        uops = lower(spec, ver=ver)
        shas[ver] = DveOpSpec(
            name=name, opcode=row, uops=uops, rd1_en=_has_src1(spec)
        ).sha(ver)
    op = DveOp(name, spec, subdim=False, uops_sha=shas)
    dve_ops_mod.OPS.append(op)
    dve_ops_mod.CUSTOM_DVE_SPECS[name] = spec
    _GATE_LUT_OP = op
    return op


# ---------------------------------------------------------------------------
# Device program builder (parameterized so a small version can be simulated)
# ---------------------------------------------------------------------------


def build_nc(bs=BS, n=N, g=G, group=8, ncores=NCORES):
    """One SPMD program; all cores run it on their own batch shard."""
    lut_op = _register_gate_lut()
    nblk = g // 128          # gate blocks of 128
    ngrp = nblk // group     # gather groups
    ntab = 2 * n + 2         # x^T rows, ~x^T rows, ones row, zeros row
    mtiles = bs // 128       # batch sub-tiles per core
    nidx = group * 128       # indices per dma_gather call
    percall = nidx // 16     # int16s per partition per call

    nc = bacc.Bacc(
        "TRN2", target_bir_lowering=False, debug=False, num_devices=ncores
    )
    tab = nc.dram_tensor("tab", [ntab, bs], mybir.dt.uint8, kind="ExternalInput")
    idxs = nc.dram_tensor(
        "idxs", [128, 2 * ngrp * percall], mybir.dt.int16, kind="ExternalInput"
    )
    cst = nc.dram_tensor("cst", [128, 2 * nblk], mybir.dt.float32, kind="ExternalInput")
    ident = nc.dram_tensor("ident", [128, 128], mybir.dt.bfloat16, kind="ExternalInput")
    outd = nc.dram_tensor("out", [bs, g], mybir.dt.uint8, kind="ExternalOutput")

    with tile.TileContext(nc) as tc, ExitStack() as ctx:
        pconst = ctx.enter_context(tc.tile_pool(name="const", bufs=1))
        pgather = ctx.enter_context(tc.tile_pool(name="gather", bufs=2))
        pl = ctx.enter_context(tc.tile_pool(name="lut", bufs=2))
        posb = ctx.enter_context(tc.tile_pool(name="osb", bufs=2))
        pps = ctx.enter_context(tc.tile_pool(name="ps", bufs=4, space="PSUM"))

        idx_t = pconst.tile([128, idxs.shape[1]], mybir.dt.int16)
        nc.sync.dma_start(idx_t[:], idxs[:])
        cst_t = pconst.tile([128, 2 * nblk], mybir.dt.float32)
        nc.sync.dma_start(cst_t[:], cst[:])
        id_t = pconst.tile([128, 128], mybir.dt.bfloat16)
        nc.sync.dma_start(id_t[:], ident[:])

        for gi in range(ngrp):
            a_t = pgather.tile([128, group, bs], mybir.dt.uint8, tag="a")
            b_t = pgather.tile([128, group, bs], mybir.dt.uint8, tag="b")
            off = gi * 2 * percall
            nc.gpsimd.dma_gather(
                a_t[:],
                tab[:],
                idx_t[:, off : off + percall],
                nidx,
                nidx,
                bs,
                single_packet=False,
            )
            nc.gpsimd.dma_gather(
                b_t[:],
                tab[:],
                idx_t[:, off + percall : off + 2 * percall],
                nidx,
                nidx,
                bs,
                single_packet=False,
            )
            ls = []
            for j in range(group):
                bk = gi * group + j
                l_t = pl.tile([128, bs], mybir.dt.bfloat16, tag=f"l{j}")
                nc.vector._custom_dve(
                    lut_op,
                    out=l_t[:],
                    in0=a_t[:, j, :],
                    in1=b_t[:, j, :],
                    s0=cst_t[:, bk : bk + 1],
                    s1=cst_t[:, nblk + bk : nblk + bk + 1],
                )
                ls.append(l_t)
            for m in range(mtiles):
                osb = posb.tile([128, group * 128], mybir.dt.uint8, tag=f"o{m}")
                ps = pps.tile([128, group * 128], mybir.dt.bfloat16)
                for j in range(group):
                    nc.tensor.transpose(
                        ps[:, j * 128 : (j + 1) * 128],
                        ls[j][:, m * 128 : (m + 1) * 128],
                        id_t[:],
                    )
                nc.scalar.activation(
                    osb[:], ps[:], mybir.ActivationFunctionType.Copy
                )
                nc.sync.dma_start(
                    outd[
                        m * 128 : (m + 1) * 128,
                        gi * group * 128 : (gi + 1) * group * 128,
                    ],
                    osb[:],
                )
    nc.compile()
    return nc


# ---------------------------------------------------------------------------
# Host-side input prep
# ---------------------------------------------------------------------------


def _prep_inputs(x, gates, choices, bs=BS, n=N, g=G, group=8, ncores=NCORES):
    nblk = g // 128
    ngrp = nblk // group
    x8 = np.asarray(x, dtype=np.uint8)
    gates8 = np.asarray(gates, dtype=np.uint8)
    ch = np.asarray(choices, dtype=np.int64)

    tt = (gates8 << np.arange(4, dtype=np.uint8)).sum(axis=1).astype(np.int64)
    fa, fb, t1, t2 = (_FORMS[tt, k] for k in range(4))

    # operand row index in TAB for each gate
    ia = np.where(fa <= 1, ch[:, 0] + fa * n, 2 * n + (fa - 2))
    ib = np.where(fb <= 1, ch[:, 1] + fb * n, 2 * n + (fb - 2))
    assert ia.max() < 2 * n + 2 and ib.max() < 2 * n + 2

    # dma_gather wrapped index layout: per call, idx i -> partition i%16,
    # slot i//16; replicated across the 8 gpsimd cores (x8 partitions).
    cols = []
    for gi in range(ngrp):
        for arr in (ia, ib):
            flat = arr[gi * group * 128 : (gi + 1) * group * 128].astype(np.int16)
            wrapped = flat.reshape(-1, 16).T  # [16, nidx/16]
            cols.append(np.tile(wrapped, (8, 1)))  # [128, nidx/16]
    idxs_np = np.ascontiguousarray(np.concatenate(cols, axis=1))

    # thresholds, [128, 2*nblk] f32; column bk = t1 of gates bk*128..bk*128+127
    t1m = t1.reshape(nblk, 128).T.astype(np.float32)
    t2m = t2.reshape(nblk, 128).T.astype(np.float32)
    cst_np = np.ascontiguousarray(np.concatenate([t1m, t2m], axis=1))

    ident_np = np.eye(128, dtype=ml_dtypes.bfloat16)

    # doubled table
    xt = x8.T  # [n, B] view
    ntab = 2 * n + 2
    in_maps = []
    for k in range(ncores):
        sl = slice(k * bs, (k + 1) * bs)
        tabk = np.empty((ntab, bs), dtype=np.uint8)
        tabk[:n] = xt[:, sl]
        tabk[n : 2 * n] = 1 - tabk[:n]
        tabk[2 * n] = 1
        tabk[2 * n + 1] = 0
        in_maps.append(
            {"tab": tabk, "idxs": idxs_np, "cst": cst_np, "ident": ident_np}
        )
    return in_maps


# ---------------------------------------------------------------------------
# Entry point
# ---------------------------------------------------------------------------

_NC_CACHE = {}


def _get_nc(key=(BS, N, G, 8)):
    if key not in _NC_CACHE:
        _NC_CACHE[key] = build_nc(*key)
    return _NC_CACHE[key]


def kernel(x, gates, choices):
    in_maps = _prep_inputs(x, gates, choices)
    nc = _get_nc()
    res = run_bass_kernel_spmd(nc, in_maps, list(range(NCORES)))
    out = np.concatenate([res.results[k]["out"] for k in range(NCORES)], axis=0)
    return out.astype(bool)



# revision 2
# speedup vs baseline: 5.5552x; 5.5552x over previous
"""GateRow kernel for Trainium2 (8 NeuronCores, SPMD gate-parallel).

Problem: out[b, g] = gates[g, 2*x[b, c0[g]] + x[b, c1[g]]]
  x: [16384, 8192] bool, gates: [8192, 4] bool, choices: [8192, 2] int32.

Strategy: bit-pack the batch dimension (8 rows/byte) so every boolean
gate evaluates bitwise on uint8 bytes.  Every 2-input boolean function
is expressible as  P op Q  with op in {AND, OR, XOR} and P, Q rows of
the table [x_packed^T ; ~x_packed^T ; zeros ; ones] (input inversion /
constants folded into the row choice).  Gates are sharded across the 8
cores and sorted by op-class into 128-gate blocks so each core runs
exactly eight bitwise tensor_tensor instructions over gathered rows.

Per core: dma_gather 2 x 1024 rows of 2048B (4 MiB), 8 DVE bitwise ops
over [128, 2048] tiles, DMA out 2 MiB of packed results.  Host side:
pack bits, build the table, classify/sort gates, and unpack + transpose
the packed output.
"""

import sys

for _p in ("/opt/trn_rl_repo", "/opt/pypackages"):
    if _p not in sys.path:
        sys.path.append(_p)

from contextlib import ExitStack

import numpy as np

import concourse.bacc as bacc
import concourse.tile as tile
import concourse.mybir as mybir
from concourse.bass_utils import run_bass_kernel_spmd

B, N, G, NCORES = 16384, 8192, 8192, 8
GPC = G // NCORES          # 1024 gate slots per core
PB = B // 8                # 2048 packed bytes per table row
NTAB = 2 * N + 2           # x rows, ~x rows, zeros row, ones row
NBLK = GPC // 128          # 8 blocks of 128 gates per core
NQ = 4                     # gather calls per stream (pipeline granularity)
QI = GPC // NQ             # 256 indices per gather call

# ---------------------------------------------------------------------------
# Gate classification: truth table tt (bit i = gates[g, i], i = 2a+b) ->
# per class c in {0:AND, 1:OR, 2:XOR}: sections (psec, qsec) with
# sec in {0:x[c0], 1:~x[c0], 2:x[c1], 3:~x[c1], 4:zeros, 5:ones}
# such that f(a, b) == sec_p op_c sec_q, or None if not expressible.
# ---------------------------------------------------------------------------


def _forms():
    forms = [[None] * 3 for _ in range(16)]
    for tt in range(16):
        for cls in range(3):
            for ps in range(6):
                for qs in range(6):
                    ok = True
                    for a in (0, 1):
                        for b in (0, 1):
                            va = (a, 1 - a, b, 1 - b, 0, 1)[ps]
                            vb = (a, 1 - a, b, 1 - b, 0, 1)[qs]
                            f = (va & vb, va | vb, va ^ vb)[cls]
                            if f != ((tt >> (2 * a + b)) & 1):
                                ok = False
                    if ok and forms[tt][cls] is None:
                        forms[tt][cls] = (ps, qs)
    return forms


_FORMS = _forms()


# ---------------------------------------------------------------------------
# Device program: per core, gather P/Q rows and apply per-block bitwise op.
# Block classes are uniform across cores: blocks [0, na) AND, [na, na+no)
# OR, [na+no, 8) XOR.
# ---------------------------------------------------------------------------


def build_nc(na, no, nx):
    assert na + no + nx == NBLK
    nc = bacc.Bacc(
        "TRN2", target_bir_lowering=False, debug=False, num_devices=NCORES
    )
    tab = nc.dram_tensor("tab", [NTAB, PB], mybir.dt.uint8, kind="ExternalInput")
    idxs = nc.dram_tensor(
        "idxs", [128, 2 * GPC // 16], mybir.dt.int16, kind="ExternalInput"
    )
    outd = nc.dram_tensor("out", [GPC, PB], mybir.dt.uint8, kind="ExternalOutput")

    ops = (
        [mybir.AluOpType.bitwise_and] * na
        + [mybir.AluOpType.bitwise_or] * no
        + [mybir.AluOpType.bitwise_xor] * nx
    )
    bpq = QI // 128  # gate blocks per gather call (2)

    with tile.TileContext(nc) as tc, ExitStack() as ctx:
        pconst = ctx.enter_context(tc.tile_pool(name="const", bufs=1))
        pdata = ctx.enter_context(tc.tile_pool(name="data", bufs=1))

        idx_t = pconst.tile([128, 2 * GPC // 16], mybir.dt.int16)
        nc.sync.dma_start(idx_t[:], idxs[:])

        p_t = pdata.tile([128, NBLK, PB], mybir.dt.uint8)
        q_t = pdata.tile([128, NBLK, PB], mybir.dt.uint8)
        lut = pdata.tile([128, NBLK, PB], mybir.dt.uint8)

        for q in range(NQ):
            for off, t in ((0, p_t), (GPC // 16, q_t)):
                nc.gpsimd.dma_gather(
                    t[:, q * bpq : (q + 1) * bpq, :],
                    tab[:],
                    idx_t[:, off + q * (QI // 16) : off + (q + 1) * (QI // 16)],
                    QI,
                    QI,
                    PB,
                    single_packet=False,
                )
            for j in range(q * bpq, (q + 1) * bpq):
                nc.vector.tensor_tensor(
                    lut[:, j, :], p_t[:, j, :], q_t[:, j, :], ops[j]
                )
                nc.sync.dma_start(outd[j * 128 : (j + 1) * 128, :], lut[:, j, :])
    nc.compile()
    return nc


_NC_CACHE = {}


def _get_nc(key):
    if key not in _NC_CACHE:
        _NC_CACHE[key] = build_nc(*key)
    return _NC_CACHE[key]


# ---------------------------------------------------------------------------
# Host-side planning: classify gates, choose uniform block allocation,
# deal gates to (core, slot), build index arrays.
# ---------------------------------------------------------------------------


def _plan(gates, choices):
    gates8 = np.asarray(gates, dtype=np.uint8)
    ch = np.asarray(choices, dtype=np.int64)
    tt = (gates8 << np.arange(4, dtype=np.uint8)).sum(axis=1).astype(np.int64)

    can = np.array(
        [[_FORMS[t][c] is not None for c in range(3)] for t in range(16)]
    )[tt]  # [G, 3]
    ncls = can.sum(axis=1)
    strict = [np.where(can[:, c] & (ncls == 1))[0] for c in range(3)]
    flex = np.where(ncls > 1)[0]
    assert (ncls > 0).all()

    # per-core block allocation (uniform across cores)
    per_core_strict = [int(np.ceil(len(s) / NCORES)) for s in strict]
    nx = max(1, int(np.ceil(per_core_strict[2] / 128))) if per_core_strict[2] else 0
    no = max(1, int(np.ceil(per_core_strict[1] / 128))) if per_core_strict[1] else 0
    na = NBLK - no - nx
    assert na * 128 >= per_core_strict[0] and na >= 0, (na, no, nx)

    caps = [na * 128, no * 128, nx * 128]

    # deal strict gates round-robin, then fill leftover capacity with flex
    slots = np.full((NCORES, 3), 0, dtype=np.int64)  # used per class
    assign = [[[] for _ in range(3)] for _ in range(NCORES)]
    for c in range(3):
        for i, g in enumerate(strict[c]):
            k = i % NCORES
            assign[k][c].append(g)
            slots[k, c] += 1
    assert (slots <= np.array(caps)[None, :]).all()
    fi = 0
    flex = list(flex)
    for k in range(NCORES):
        for c in range(3):
            while slots[k, c] < caps[c] and fi < len(flex):
                assign[k][c].append(flex[fi])
                slots[k, c] += 1
                fi += 1
    assert fi == len(flex)
    assert (slots == np.array(caps)[None, :]).all()

    # per-slot gate ids and P/Q row indices
    psec_tab = np.zeros((16, 3), dtype=np.int64)
    qsec_tab = np.zeros((16, 3), dtype=np.int64)
    for t in range(16):
        for c in range(3):
            if _FORMS[t][c] is not None:
                psec_tab[t, c], qsec_tab[t, c] = _FORMS[t][c]

    g_of_slot = np.empty((NCORES, GPC), dtype=np.int64)
    idx_maps = []
    for k in range(NCORES):
        gk = np.concatenate(
            [np.asarray(assign[k][c], dtype=np.int64) for c in range(3)]
        )
        assert gk.shape == (GPC,)
        g_of_slot[k] = gk
        cls_of_slot = np.repeat([0, 1, 2], [na * 128, no * 128, nx * 128])
        ps = psec_tab[tt[gk], cls_of_slot]
        qs = qsec_tab[tt[gk], cls_of_slot]
        c0, c1 = ch[gk, 0], ch[gk, 1]

        def rows(sec):
            return np.select(
                [sec == 0, sec == 1, sec == 2, sec == 3, sec == 4, sec == 5],
                [c0, N + c0, c1, N + c1,
                 np.full(GPC, 2 * N), np.full(GPC, 2 * N + 1)],
            )

        cols = []
        for arr in (rows(ps), rows(qs)):
            for q in range(NQ):
                flat = arr[q * QI : (q + 1) * QI].astype(np.int16)
                cols.append(np.tile(flat.reshape(-1, 16).T, (8, 1)))
        idx_maps.append(np.ascontiguousarray(np.concatenate(cols, axis=1)))

    return (na, no, nx), g_of_slot, idx_maps


def _build_tab(x):
    x8 = np.asarray(x, dtype=np.uint8)
    xp = np.packbits(x8, axis=0)              # [PB, N]
    tab = np.empty((NTAB, PB), dtype=np.uint8)
    tab[:N] = xp.T
    tab[N : 2 * N] = 255 - tab[:N]
    tab[2 * N] = 0
    tab[2 * N + 1] = 255
    return tab


# ---------------------------------------------------------------------------
# Entry point
# ---------------------------------------------------------------------------

_PLAN_CACHE = {}


def _get_plan(gates, choices):
    key = (gates.tobytes(), choices.tobytes())
    h = hash(key)
    if h not in _PLAN_CACHE:
        _PLAN_CACHE[h] = _plan(gates, choices)
    return _PLAN_CACHE[h]


def kernel(x, gates, choices):
    aox, g_of_slot, idx_maps = _get_plan(
        np.asarray(gates), np.asarray(choices)
    )
    tab = _build_tab(x)
    nc = _get_nc(aox)
    in_maps = [{"tab": tab, "idxs": idx_maps[k]} for k in range(NCORES)]
    res = run_bass_kernel_spmd(nc, in_maps, list(range(NCORES)))

    packed = np.empty((G, PB), dtype=np.uint8)
    for k in range(NCORES):
        packed[g_of_slot[k]] = res.results[k]["out"]
    out = np.unpackbits(np.ascontiguousarray(packed.T), axis=0)
    return out.view(np.bool_)


# revision 3
# speedup vs baseline: 5.7459x; 1.0343x over previous
"""GateRow kernel for Trainium2 (8 NeuronCores, SPMD gate-parallel).

Problem: out[b, g] = gates[g, 2*x[b, c0[g]] + x[b, c1[g]]]
  x: [16384, 8192] bool, gates: [8192, 4] bool, choices: [8192, 2] int32.

Strategy: bit-pack the batch dimension (8 rows/byte) so every boolean
gate evaluates bitwise on uint8 bytes.  Every 2-input boolean function
is expressible as  P op Q  with op in {AND, OR, XOR} and P, Q rows of
the table [x_packed^T ; ~x_packed^T ; zeros ; ones] (input inversion /
constants folded into the row choice).  Gates are sharded across the 8
cores and sorted by op-class into 128-gate blocks so each core runs
exactly eight bitwise tensor_tensor instructions over gathered rows.

Per core: dma_gather 2 x 1024 rows of 2048B (4 MiB), 8 DVE bitwise ops
over [128, 2048] tiles, DMA out 2 MiB of packed results.  Host side:
pack bits, build the table, classify/sort gates, and unpack + transpose
the packed output.
"""

import sys

for _p in ("/opt/trn_rl_repo", "/opt/pypackages"):
    if _p not in sys.path:
        sys.path.append(_p)

from contextlib import ExitStack

import numpy as np

import concourse.bacc as bacc
import concourse.tile as tile
import concourse.mybir as mybir
from concourse.bass_utils import run_bass_kernel_spmd

B, N, G, NCORES = 16384, 8192, 8192, 8
GPC = G // NCORES          # 1024 gate slots per core
PB = B // 8                # 2048 packed bytes per table row
NTAB = 2 * N + 2           # x rows, ~x rows, zeros row, ones row
NBLK = GPC // 128          # 8 blocks of 128 gates per core
NQ = 2                     # gather calls per stream (pipeline granularity)
QI = GPC // NQ             # 256 indices per gather call

# ---------------------------------------------------------------------------
# Gate classification: truth table tt (bit i = gates[g, i], i = 2a+b) ->
# per class c in {0:AND, 1:OR, 2:XOR}: sections (psec, qsec) with
# sec in {0:x[c0], 1:~x[c0], 2:x[c1], 3:~x[c1], 4:zeros, 5:ones}
# such that f(a, b) == sec_p op_c sec_q, or None if not expressible.
# ---------------------------------------------------------------------------


def _forms():
    forms = [[None] * 3 for _ in range(16)]
    for tt in range(16):
        for cls in range(3):
            for ps in range(6):
                for qs in range(6):
                    ok = True
                    for a in (0, 1):
                        for b in (0, 1):
                            va = (a, 1 - a, b, 1 - b, 0, 1)[ps]
                            vb = (a, 1 - a, b, 1 - b, 0, 1)[qs]
                            f = (va & vb, va | vb, va ^ vb)[cls]
                            if f != ((tt >> (2 * a + b)) & 1):
                                ok = False
                    if ok and forms[tt][cls] is None:
                        forms[tt][cls] = (ps, qs)
    return forms


_FORMS = _forms()


# ---------------------------------------------------------------------------
# Device program: per core, gather P/Q rows and apply per-block bitwise op.
# Block classes are uniform across cores: blocks [0, na) AND, [na, na+no)
# OR, [na+no, 8) XOR.
# ---------------------------------------------------------------------------


def build_nc(na, no, nx):
    assert na + no + nx == NBLK
    nc = bacc.Bacc(
        "TRN2", target_bir_lowering=False, debug=False, num_devices=NCORES
    )
    tab = nc.dram_tensor(
        "tab", [NTAB, PB // 2], mybir.dt.uint16, kind="ExternalInput"
    )
    idxs = nc.dram_tensor(
        "idxs", [128, 2 * GPC // 16], mybir.dt.int16, kind="ExternalInput"
    )
    outd = nc.dram_tensor(
        "out", [GPC, PB // 2], mybir.dt.uint16, kind="ExternalOutput"
    )

    ops = (
        [mybir.AluOpType.bitwise_and] * na
        + [mybir.AluOpType.bitwise_or] * no
        + [mybir.AluOpType.bitwise_xor] * nx
    )
    bpq = QI // 128  # gate blocks per gather call (2)

    with tile.TileContext(nc) as tc, ExitStack() as ctx:
        pconst = ctx.enter_context(tc.tile_pool(name="const", bufs=1))
        pdata = ctx.enter_context(tc.tile_pool(name="data", bufs=1))

        idx_t = pconst.tile([128, 2 * GPC // 16], mybir.dt.int16)
        nc.sync.dma_start(idx_t[:], idxs[:])

        p_t = pdata.tile([128, NBLK, PB // 2], mybir.dt.uint16)
        q_t = pdata.tile([128, NBLK, PB // 2], mybir.dt.uint16)
        lut = pdata.tile([128, NBLK, PB // 2], mybir.dt.uint16)

        for q in range(NQ):
            for off, t in ((0, p_t), (GPC // 16, q_t)):
                nc.gpsimd.dma_gather(
                    t[:, q * bpq : (q + 1) * bpq, :],
                    tab[:],
                    idx_t[:, off + q * (QI // 16) : off + (q + 1) * (QI // 16)],
                    QI,
                    QI,
                    PB // 2,
                    single_packet=False,
                )
            for j in range(q * bpq, (q + 1) * bpq):
                nc.vector.tensor_tensor(
                    lut[:, j, :], p_t[:, j, :], q_t[:, j, :], ops[j]
                )
                nc.sync.dma_start(outd[j * 128 : (j + 1) * 128, :], lut[:, j, :])
    nc.compile()
    return nc


_NC_CACHE = {}


def _get_nc(key):
    if key not in _NC_CACHE:
        _NC_CACHE[key] = build_nc(*key)
    return _NC_CACHE[key]


# ---------------------------------------------------------------------------
# Host-side planning: classify gates, choose uniform block allocation,
# deal gates to (core, slot), build index arrays.
# ---------------------------------------------------------------------------


def _plan(gates, choices):
    gates8 = np.asarray(gates, dtype=np.uint8)
    ch = np.asarray(choices, dtype=np.int64)
    tt = (gates8 << np.arange(4, dtype=np.uint8)).sum(axis=1).astype(np.int64)

    can = np.array(
        [[_FORMS[t][c] is not None for c in range(3)] for t in range(16)]
    )[tt]  # [G, 3]
    ncls = can.sum(axis=1)
    strict = [np.where(can[:, c] & (ncls == 1))[0] for c in range(3)]
    flex = np.where(ncls > 1)[0]
    assert (ncls > 0).all()

    # per-core block allocation (uniform across cores)
    per_core_strict = [int(np.ceil(len(s) / NCORES)) for s in strict]
    nx = max(1, int(np.ceil(per_core_strict[2] / 128))) if per_core_strict[2] else 0
    no = max(1, int(np.ceil(per_core_strict[1] / 128))) if per_core_strict[1] else 0
    na = NBLK - no - nx
    assert na * 128 >= per_core_strict[0] and na >= 0, (na, no, nx)

    caps = [na * 128, no * 128, nx * 128]

    # deal strict gates round-robin, then fill leftover capacity with flex
    slots = np.full((NCORES, 3), 0, dtype=np.int64)  # used per class
    assign = [[[] for _ in range(3)] for _ in range(NCORES)]
    for c in range(3):
        for i, g in enumerate(strict[c]):
            k = i % NCORES
            assign[k][c].append(g)
            slots[k, c] += 1
    assert (slots <= np.array(caps)[None, :]).all()
    fi = 0
    flex = list(flex)
    for k in range(NCORES):
        for c in range(3):
            while slots[k, c] < caps[c] and fi < len(flex):
                assign[k][c].append(flex[fi])
                slots[k, c] += 1
                fi += 1
    assert fi == len(flex)
    assert (slots == np.array(caps)[None, :]).all()

    # per-slot gate ids and P/Q row indices
    psec_tab = np.zeros((16, 3), dtype=np.int64)
    qsec_tab = np.zeros((16, 3), dtype=np.int64)
    for t in range(16):
        for c in range(3):
            if _FORMS[t][c] is not None:
                psec_tab[t, c], qsec_tab[t, c] = _FORMS[t][c]

    g_of_slot = np.empty((NCORES, GPC), dtype=np.int64)
    idx_maps = []
    for k in range(NCORES):
        gk = np.concatenate(
            [np.asarray(assign[k][c], dtype=np.int64) for c in range(3)]
        )
        assert gk.shape == (GPC,)
        g_of_slot[k] = gk
        cls_of_slot = np.repeat([0, 1, 2], [na * 128, no * 128, nx * 128])
        ps = psec_tab[tt[gk], cls_of_slot]
        qs = qsec_tab[tt[gk], cls_of_slot]
        c0, c1 = ch[gk, 0], ch[gk, 1]

        def rows(sec):
            return np.select(
                [sec == 0, sec == 1, sec == 2, sec == 3, sec == 4, sec == 5],
                [c0, N + c0, c1, N + c1,
                 np.full(GPC, 2 * N), np.full(GPC, 2 * N + 1)],
            )

        cols = []
        for arr in (rows(ps), rows(qs)):
            for q in range(NQ):
                flat = arr[q * QI : (q + 1) * QI].astype(np.int16)
                cols.append(np.tile(flat.reshape(-1, 16).T, (8, 1)))
        idx_maps.append(np.ascontiguousarray(np.concatenate(cols, axis=1)))

    return (na, no, nx), g_of_slot, idx_maps


def _build_tab(x):
    x8 = np.asarray(x, dtype=np.uint8)
    xp = np.packbits(x8, axis=0)              # [PB, N]
    tab = np.empty((NTAB, PB), dtype=np.uint8)
    tab[:N] = xp.T
    tab[N : 2 * N] = 255 - tab[:N]
    tab[2 * N] = 0
    tab[2 * N + 1] = 255
    return tab.view(np.uint16)


# ---------------------------------------------------------------------------
# Entry point
# ---------------------------------------------------------------------------

_PLAN_CACHE = {}


def _get_plan(gates, choices):
    key = (gates.tobytes(), choices.tobytes())
    h = hash(key)
    if h not in _PLAN_CACHE:
        _PLAN_CACHE[h] = _plan(gates, choices)
    return _PLAN_CACHE[h]


def kernel(x, gates, choices):
    aox, g_of_slot, idx_maps = _get_plan(
        np.asarray(gates), np.asarray(choices)
    )
    tab = _build_tab(x)
    nc = _get_nc(aox)
    in_maps = [{"tab": tab, "idxs": idx_maps[k]} for k in range(NCORES)]
    res = run_bass_kernel_spmd(nc, in_maps, list(range(NCORES)))

    packed = np.empty((G, PB), dtype=np.uint8)
    for k in range(NCORES):
        packed[g_of_slot[k]] = res.results[k]["out"].view(np.uint8)
    out = np.unpackbits(np.ascontiguousarray(packed.T), axis=0)
    return out.view(np.bool_)


# revision 4
# speedup vs baseline: 6.3574x; 1.1064x over previous
"""GateRow kernel for Trainium2 (8 NeuronCores, SPMD gate-parallel).

Problem: out[b, g] = gates[g, 2*x[b, c0[g]] + x[b, c1[g]]]
  x: [16384, 8192] bool, gates: [8192, 4] bool, choices: [8192, 2] int32.

Strategy: bit-pack the batch dimension (8 rows/byte; stored as uint16
words for 2x DVE throughput) so every boolean gate evaluates bitwise.
Every 2-input boolean function is either a single table row (copies,
constants, inverses -- the table holds x, ~x, zeros, ones) or  P op Q
with op in {AND, OR, XOR} and P, Q table rows.  Gates are sharded
across the 8 cores and sorted by op-class into 128-gate blocks so each
core runs one bitwise tensor_tensor per AND/OR/XOR block and nothing at
all for COPY blocks (gathered rows stream straight back out).

Per core: dma_gather ~1.7k rows of 2048B (~3.3 MiB), ~5 DVE bitwise
ops over [128, 1024] uint16 tiles, DMA out 2 MiB of packed results.
Host side: pack bits, build the table, classify/sort gates, unpack +
transpose the packed output.
"""

import sys

for _p in ("/opt/trn_rl_repo", "/opt/pypackages"):
    if _p not in sys.path:
        sys.path.append(_p)

from contextlib import ExitStack

import numpy as np

import concourse.bacc as bacc
import concourse.tile as tile
import concourse.mybir as mybir
from concourse.bass_utils import run_bass_kernel_spmd

B, N, G, NCORES = 16384, 8192, 8192, 8
GPC = G // NCORES          # 1024 gate slots per core
PB = B // 8                # 2048 packed bytes per table row
PW = PB // 2               # 1024 uint16 words per table row
NTAB = 2 * N + 2           # x rows, ~x rows, zeros row, ones row
NBLK = GPC // 128          # 8 blocks of 128 gates per core

# ---------------------------------------------------------------------------
# Gate classification.  Truth table tt (bit i = gates[g, i], i = 2a+b).
# Classes: 0:AND  1:OR  2:XOR (two rows)   3:COPY (single row).
# Sections: 0:x[c0] 1:~x[c0] 2:x[c1] 3:~x[c1] 4:zeros 5:ones.
# ---------------------------------------------------------------------------


def _forms():
    forms = [[None] * 4 for _ in range(16)]
    for tt in range(16):
        for cls in range(3):
            for ps in range(6):
                for qs in range(6):
                    ok = True
                    for a in (0, 1):
                        for b in (0, 1):
                            va = (a, 1 - a, b, 1 - b, 0, 1)[ps]
                            vb = (a, 1 - a, b, 1 - b, 0, 1)[qs]
                            f = (va & vb, va | vb, va ^ vb)[cls]
                            if f != ((tt >> (2 * a + b)) & 1):
                                ok = False
                    if ok and forms[tt][cls] is None:
                        forms[tt][cls] = (ps, qs)
        for ps in range(6):
            ok = all(
                (a, 1 - a, b, 1 - b, 0, 1)[ps] == ((tt >> (2 * a + b)) & 1)
                for a in (0, 1)
                for b in (0, 1)
            )
            if ok and forms[tt][3] is None:
                forms[tt][3] = (ps, ps)
    return forms


_FORMS = _forms()


def _sec_rows(sec, c0, c1):
    return np.select(
        [sec == 0, sec == 1, sec == 2, sec == 3, sec == 4, sec == 5],
        [c0, N + c0, c1, N + c1,
         np.full(sec.shape, 2 * N), np.full(sec.shape, 2 * N + 1)],
    )


# ---------------------------------------------------------------------------
# Device program.  Uniform across cores: blocks [0, na) AND, [na, na+no)
# OR, [na+no, naox) XOR, [naox, 8) COPY.  Gathers run per half (blocks
# 0-3 then 4-7): a p-call for all 4 blocks, a q-call for the AOX blocks.
# ---------------------------------------------------------------------------


def build_nc(na, no, nx):
    naox = na + no + nx
    assert naox <= NBLK
    nc = bacc.Bacc(
        "TRN2", target_bir_lowering=False, debug=False, num_devices=NCORES
    )
    ncols = NBLK * 8 + naox * 8  # int16 idx columns: p-stream then q-stream
    tab = nc.dram_tensor("tab", [NTAB, PW], mybir.dt.uint16, kind="ExternalInput")
    idxs = nc.dram_tensor("idxs", [128, ncols], mybir.dt.int16, kind="ExternalInput")
    outd = nc.dram_tensor("out", [GPC, PW], mybir.dt.uint16, kind="ExternalOutput")

    ops = (
        [mybir.AluOpType.bitwise_and] * na
        + [mybir.AluOpType.bitwise_or] * no
        + [mybir.AluOpType.bitwise_xor] * nx
    )
    hb = NBLK // 2  # blocks per half

    with tile.TileContext(nc) as tc, ExitStack() as ctx:
        pconst = ctx.enter_context(tc.tile_pool(name="const", bufs=1))
        pdata = ctx.enter_context(tc.tile_pool(name="data", bufs=1))

        idx_t = pconst.tile([128, ncols], mybir.dt.int16)
        nc.sync.dma_start(idx_t[:], idxs[:])

        p_t = pdata.tile([128, NBLK, PW], mybir.dt.uint16)
        q_t = pdata.tile([128, max(naox, 1), PW], mybir.dt.uint16)
        lut = pdata.tile([128, max(naox, 1), PW], mybir.dt.uint16)

        nreg = {}
        for n in {hb * 128, *(
            128 * len([j for j in range(h * hb, (h + 1) * hb) if j < naox])
            for h in (0, 1)
        )} - {0}:
            nreg[n] = nc.gpsimd.to_reg(n)

        for h in (0, 1):
            blocks = range(h * hb, (h + 1) * hb)
            aox = [j for j in blocks if j < naox]
            npi = hb * 128
            nc.gpsimd.dma_gather(
                p_t[:, h * hb : (h + 1) * hb, :],
                tab[:],
                idx_t[:, h * (npi // 16) : (h + 1) * (npi // 16)],
                npi,
                nreg[npi],
                PW,
                single_packet=False,
            )
            if aox:
                nqi = 128 * len(aox)
                qc0 = NBLK * 8 + aox[0] * 8
                nc.gpsimd.dma_gather(
                    q_t[:, aox[0] : aox[0] + len(aox), :],
                    tab[:],
                    idx_t[:, qc0 : qc0 + nqi // 16],
                    nqi,
                    nreg[nqi],
                    PW,
                    single_packet=False,
                )
            for j in blocks:
                if j < naox:
                    nc.vector.tensor_tensor(
                        lut[:, j, :], p_t[:, j, :], q_t[:, j, :], ops[j]
                    )
                    src = lut[:, j, :]
                else:
                    src = p_t[:, j, :]
                nc.sync.dma_start(outd[j * 128 : (j + 1) * 128, :], src)
    nc.compile()
    return nc


_NC_CACHE = {}


def _get_nc(key):
    if key not in _NC_CACHE:
        _NC_CACHE[key] = build_nc(*key)
    return _NC_CACHE[key]


# ---------------------------------------------------------------------------
# Host-side planning.
# ---------------------------------------------------------------------------


def _plan(gates, choices):
    gates8 = np.asarray(gates, dtype=np.uint8)
    ch = np.asarray(choices, dtype=np.int64)
    tt = (gates8 << np.arange(4, dtype=np.uint8)).sum(axis=1).astype(np.int64)

    copyable = np.array([_FORMS[t][3] is not None for t in range(16)])[tt]
    cls_strict = np.array(
        [next(c for c in range(3) if _FORMS[t][c] is not None) for t in range(16)]
    )[tt]
    strict = [np.where(~copyable & (cls_strict == c))[0] for c in range(3)]
    copies = np.where(copyable)[0]

    # deal strict gates round-robin
    assign = [[[] for _ in range(4)] for _ in range(NCORES)]
    for c in range(3):
        for i, g in enumerate(strict[c]):
            assign[i % NCORES][c].append(g)

    maxc = [max(len(assign[k][c]) for k in range(NCORES)) for c in range(3)]
    na, no, nx = (int(np.ceil(m / 128)) for m in maxc)
    naox = na + no + nx
    assert naox <= NBLK, (na, no, nx)
    caps = [na * 128, no * 128, nx * 128]

    # copy-capable gates: pad AOX segments to caps, rest go to COPY blocks
    ci = 0
    copies = list(copies)
    for k in range(NCORES):
        for c in range(3):
            while len(assign[k][c]) < caps[c]:
                assign[k][c].append(copies[ci])
                ci += 1
        need = GPC - naox * 128
        assign[k][3] = copies[ci : ci + need]
        ci += need
    assert ci == len(copies)

    psec_tab = np.full((16, 4), -1, dtype=np.int64)
    qsec_tab = np.full((16, 4), -1, dtype=np.int64)
    for t in range(16):
        for c in range(4):
            if _FORMS[t][c] is not None:
                psec_tab[t, c], qsec_tab[t, c] = _FORMS[t][c]

    g_of_slot = np.empty((NCORES, GPC), dtype=np.int64)
    idx_maps = []
    hb = NBLK // 2
    for k in range(NCORES):
        segs, segcls = [], []
        for c in range(4):
            gk = np.asarray(assign[k][c], dtype=np.int64)
            if not len(gk):
                continue
            # sort by p-row for DRAM locality
            pr = _sec_rows(psec_tab[tt[gk], c], ch[gk, 0], ch[gk, 1])
            o = np.argsort(pr, kind="stable")
            segs.append(gk[o])
            segcls.append(np.full(len(gk), c))
        gk = np.concatenate(segs)
        cls = np.concatenate(segcls)
        assert gk.shape == (GPC,)
        g_of_slot[k] = gk
        p_rows = _sec_rows(psec_tab[tt[gk], cls], ch[gk, 0], ch[gk, 1])
        q_rows = _sec_rows(qsec_tab[tt[gk], cls], ch[gk, 0], ch[gk, 1])

        cols = []
        for h in (0, 1):
            flat = p_rows[h * hb * 128 : (h + 1) * hb * 128].astype(np.int16)
            cols.append(np.tile(flat.reshape(-1, 16).T, (8, 1)))
        naox_k = naox
        for h in (0, 1):
            lo, hi = h * hb, min((h + 1) * hb, naox_k)
            if lo < hi:
                flat = q_rows[lo * 128 : hi * 128].astype(np.int16)
                cols.append(np.tile(flat.reshape(-1, 16).T, (8, 1)))
        idx_maps.append(np.ascontiguousarray(np.concatenate(cols, axis=1)))

    return (na, no, nx), g_of_slot, idx_maps


def _build_tab(x):
    x8 = np.asarray(x, dtype=np.uint8)
    xp = np.packbits(x8, axis=0)              # [PB, N]
    tab = np.empty((NTAB, PB), dtype=np.uint8)
    tab[:N] = xp.T
    tab[N : 2 * N] = 255 - tab[:N]
    tab[2 * N] = 0
    tab[2 * N + 1] = 255
    return tab.view(np.uint16)


# ---------------------------------------------------------------------------
# Entry point
# ---------------------------------------------------------------------------

_PLAN_CACHE = {}


def _get_plan(gates, choices):
    h = hash((gates.tobytes(), choices.tobytes()))
    if h not in _PLAN_CACHE:
        _PLAN_CACHE[h] = _plan(gates, choices)
    return _PLAN_CACHE[h]


def kernel(x, gates, choices):
    aox, g_of_slot, idx_maps = _get_plan(np.asarray(gates), np.asarray(choices))
    tab = _build_tab(x)
    nc = _get_nc(aox)
    in_maps = [{"tab": tab, "idxs": idx_maps[k]} for k in range(NCORES)]
    res = run_bass_kernel_spmd(nc, in_maps, list(range(NCORES)))

    packed = np.empty((G, PB), dtype=np.uint8)
    for k in range(NCORES):
        packed[g_of_slot[k]] = res.results[k]["out"].view(np.uint8)
    out = np.unpackbits(np.ascontiguousarray(packed.T), axis=0)
    return out.view(np.bool_)
